# revision 34
# baseline (speedup 1.0000x reference)
"""Trainium2 Bass kernel for nn_DeformableCrossAttention.

Sharding: data-parallel over batch B=8 across 8 NeuronCores (one sample per
core).  Inside each core:
  - offset MLP in 3-pass bf16 hi/lo split (position precision matters:
    output error ~ 1.4x the position error in pixels; 3-pass keeps it ~2e-5)
  - attention MLP + out-projection in 1-pass bf16
  - v = context @ Wv in 1-pass bf16, stored bf16 in DRAM
  - bilinear sampling via per-head SWDGE dma_gather of 512B chunks, each
    covering FOUR spatial positions (even base e=floor(lin/2), parity of the
    true x0 folded into the DVE corner weights t4)
  - attention-weighted bilinear reduce on DVE in bf16 (4 x-slots per corner
    row, zero weight on the two unused slots)
  - output emitted transposed; host transposes back.

All weight/x streams host-packed into contiguous per-m-tile layouts so every
DMA is >=2KB-run contiguous.  Self-contained: hardcodes all problem shapes.
"""
import sys
sys.path.insert(0, "/opt/trn_rl_repo")

import numpy as np
import concourse.bass as bass
import concourse.mybir as mybir
import concourse.tile as tile
from concourse import bacc
from concourse.bass_utils import run_bass_kernel_spmd
from concourse.masks import make_identity

F32 = mybir.dt.float32
BF16 = mybir.dt.bfloat16
I16 = mybir.dt.int16
I32 = mybir.dt.int32
AF = mybir.ActivationFunctionType
ALU = mybir.AluOpType
AX = mybir.AxisListType

B, N, DIM = 8, 256, 1024
HEADS, DH, P = 16, 64, 8
HS = WS = 64
CTX = HS * WS            # 4096
INNER = HEADS * DH       # 1024
KT = DIM // 128          # 8 k-tiles
PLANE = CTX * DH         # per-head v plane elements (262144, bf16)

CTX_SUP = 512            # ctx supertile rows
N_SUP = CTX // CTX_SUP   # 8 supertiles
M_PER_SUP = CTX_SUP // 128

_CACHE = {}


def _ap(t, offset, dims):
    return bass.AP(t.ap().tensor if hasattr(t, "ap") else t.tensor, offset, dims)


def _sap(tile_obj, extra, dims):
    """Sub-AP of an SBUF tile: keep its partition dim, custom free dims,
    extra offset in elements."""
    a = tile_obj[:]
    return bass.AP(a.tensor, a.offset + extra, [list(a.ap[0])] + dims)


def _build(repeat=1, stages=3):
    nc = bacc.Bacc("TRN2", target_bir_lowering=False, debug=False,
                   num_swdge_queues=4)

    # ---------------- I/O (host-packed layouts) ----------------
    # ctxpk[sup*128+p, k*512+c] = bf16(ctxT)[k*128+p, sup*512+c]
    ctxpk = nc.dram_tensor("ctxpk", [N_SUP * 128, KT * CTX_SUP], BF16,
                           kind="ExternalInput")
    Wv = nc.dram_tensor("Wv", [DIM, INNER], BF16, kind="ExternalInput")
    # xoffpk[p, lvl*2048 + k*256 + n] (lvl 0=hi, 1=lo)
    xoffpk = nc.dram_tensor("xoffpk", [128, 2 * KT * N], BF16,
                            kind="ExternalInput")
    xattpk = nc.dram_tensor("xattpk", [128, KT * N], BF16, kind="ExternalInput")
    # Wo1p[m*128+p, lvl*1024 + k*128+j]
    Wo1p = nc.dram_tensor("Wo1p", [KT * 128, 2 * DIM], BF16,
                          kind="ExternalInput")
    # Wo2pk[p, lvl*2048 + k*256 + c]
    Wo2pk = nc.dram_tensor("Wo2pk", [128, 2 * KT * 256], BF16,
                           kind="ExternalInput")
    # Wa1p[m*128+p, k*128+j]
    Wa1p = nc.dram_tensor("Wa1p", [KT * 128, DIM], BF16, kind="ExternalInput")
    # Wa2pk[p, k*128+c]
    Wa2pk = nc.dram_tensor("Wa2pk", [128, KT * 128], BF16, kind="ExternalInput")
    # Woutp[m*128+p, k*128+j]
    Woutp = nc.dram_tensor("Woutp", [KT * 128, DIM], BF16, kind="ExternalInput")
    # bpack[p, col]: 0:8 b_off1, 8:10 b_off2p, 10:18 b_att1, 18 b_att2,
    # 19:27 b_out
    bpack = nc.dram_tensor("bpack", [128, 27], F32, kind="ExternalInput")

    outT = nc.dram_tensor("outT", [DIM, N], F32, kind="ExternalOutput")

    # DRAM scratch: bf16 quad layout [slot, head, 4, DH]: slot s holds the
    # full 2x2 bilinear footprint [v[s], v[s+1], v[s+64], v[s+65]] per head
    # (512B chunks, 8KB stride) so ONE gather descriptor covers all 4 corners
    v_dram = nc.dram_tensor("v_dram", [CTX, HEADS, 4, DH], BF16)
    # idxC[q, h, p, m] int16
    idxC = nc.dram_tensor("idxC", [16, HEADS, P, 16], I16)

    vwrite_insts = []
    cwrite_insts = []
    gather_insts = []
    idxload_insts = []

    with tile.TileContext(nc) as tc:
        import contextlib
        with contextlib.ExitStack() as ctx:
            persist = ctx.enter_context(tc.tile_pool(name="persist", bufs=1))
            ws = ctx.enter_context(tc.tile_pool(name="wstream", bufs=2))
            h1p = ctx.enter_context(tc.tile_pool(name="h1p", bufs=1))
            ctxp = ctx.enter_context(tc.tile_pool(name="ctxp", bufs=2))
            vsbp = ctx.enter_context(tc.tile_pool(name="vsbp", bufs=2))
            gp = ctx.enter_context(tc.tile_pool(name="gp", bufs=8))
            wtp = ctx.enter_context(tc.tile_pool(name="wtp", bufs=2))
            scr = ctx.enter_context(tc.tile_pool(name="scr", bufs=1))
            mps = ctx.enter_context(tc.tile_pool(name="mps", bufs=2, space="PSUM"))
            vps = ctx.enter_context(tc.tile_pool(name="vps", bufs=2, space="PSUM"))
            tps = ctx.enter_context(tc.tile_pool(name="tps", bufs=2, space="PSUM"))
            if repeat > 1:
                ctx.enter_context(tc.For_i(0, repeat, 1))

            # ---------- persistent loads ----------
            def load_tiles(dram, rows, cols, dt, tag):
                ts_ = []
                for k in range(rows // 128):
                    t = persist.tile([128, cols], dt, tag=f"{tag}_{k}")
                    nc.sync.dma_start(t[:], dram[k * 128:(k + 1) * 128, :])
                    ts_.append(t)
                return ts_

            wv = load_tiles(Wv, DIM, INNER, BF16, "wv")

            def load_packed(dram, cols, tag):
                big = persist.tile([128, cols], BF16, tag=tag, name=tag)
                nc.sync.dma_start(big[:], dram[:, :])
                return big

            wo2big = load_packed(Wo2pk, 2 * KT * 256, "wo2big")
            woff2_hi = [wo2big[:, k * 256:(k + 1) * 256] for k in range(KT)]
            woff2_lo = [wo2big[:, 2048 + k * 256:2048 + (k + 1) * 256]
                        for k in range(KT)]
            wa2big = load_packed(Wa2pk, KT * 128, "wa2big")
            watt2 = [wa2big[:, k * 128:(k + 1) * 128] for k in range(KT)]

            ball = persist.tile([128, 27], F32, tag="ball")
            nc.sync.dma_start(ball[:], bpack[:, :])
            bo1 = ball[:, 0:8]
            bo2 = ball[:, 8:10]
            ba1 = ball[:, 10:18]
            ba2 = ball[:, 18:19]
            bo = ball[:, 19:27]

            ident = persist.tile([128, 128], F32, tag="ident")
            make_identity(nc, ident[:])

            # ---------- MLP layer 1: stream packed W, psum-accumulate ----------
            def mlp_layer(w_dram, x_tile_lists, bias_tile, bcol0, mtiles, act,
                          out_tag, pool, out_dt=F32, three_pass=False):
                wcols = 2 * DIM if three_pass else DIM
                outs = []
                for m in range(mtiles):
                    wt = ws.tile([128, wcols], BF16, tag="wst",
                                 name=f"wst_{out_tag}_{m}")
                    nc.sync.dma_start(wt[:], w_dram[m * 128:(m + 1) * 128, :])
                    if three_pass:
                        whi = [wt[:, k * 128:(k + 1) * 128] for k in range(KT)]
                        wlo = [wt[:, DIM + k * 128:DIM + (k + 1) * 128]
                               for k in range(KT)]
                        passes = [(whi, x_tile_lists[0]),
                                  (whi, x_tile_lists[1]),
                                  (wlo, x_tile_lists[0])]
                    else:
                        whi = [wt[:, k * 128:(k + 1) * 128] for k in range(KT)]
                        passes = [(whi, x_tile_lists[0])]
                    ps = mps.tile([128, N], F32, tag="mlp_ps")
                    np_ = len(passes)
                    for pi, (wl, xl) in enumerate(passes):
                        for k in range(KT):
                            nc.tensor.matmul(ps[:], wl[k], xl[k],
                                             start=(pi == 0 and k == 0),
                                             stop=(pi == np_ - 1 and k == KT - 1))
                    o = pool.tile([128, N], out_dt, tag=f"{out_tag}_{m}")
                    nc.scalar.activation(o[:], ps[:], act,
                                         bias=bias_tile[:, bcol0 + m:bcol0 + m + 1])
                    outs.append(o)
                return outs

            def mlp_layer2(w_hi, w_lo, x_hi, x_lo, bias_tile, bcol0, mtiles,
                           act, out_tag, use_dve_bias=False):
                if w_lo is not None:
                    passes = [(w_hi, x_hi), (w_lo, x_hi), (w_hi, x_lo)]
                else:
                    passes = [(w_hi, x_hi)]
                outs = []
                for m in range(mtiles):
                    ps = mps.tile([128, N], F32, tag="mlp_ps")
                    np_ = len(passes)
                    for pi, (wl, xl) in enumerate(passes):
                        for k in range(KT):
                            nc.tensor.matmul(
                                ps[:], wl[k][:, m * 128:(m + 1) * 128], xl[k][:],
                                start=(pi == 0 and k == 0),
                                stop=(pi == np_ - 1 and k == KT - 1))
                    o = scr.tile([128, N], F32, tag=f"{out_tag}_{m}")
                    if use_dve_bias:
                        nc.vector.tensor_scalar(
                            o[:], ps[:], bias_tile[:, bcol0 + m:bcol0 + m + 1],
                            None, op0=ALU.add)
                    else:
                        nc.scalar.activation(
                            o[:], ps[:], act,
                            bias=bias_tile[:, bcol0 + m:bcol0 + m + 1])
                    outs.append(o)
                return outs

            # ---------- stage A: offset MLP (3-pass bf16) ----------
            def load_xpk(dram, cols, name):
                big = h1p.tile([128, cols], BF16, tag=name, name=name)
                nc.sync.dma_start(big[:], dram[:, :])
                return big

            xobig = load_xpk(xoffpk, 2 * KT * N, "xobig")
            xoff_hi = [xobig[:, k * N:(k + 1) * N] for k in range(KT)]
            xoff_lo = [xobig[:, KT * N + k * N:KT * N + (k + 1) * N]
                       for k in range(KT)]
            h1 = mlp_layer(Wo1p, [xoff_hi, xoff_lo], ball, 0, KT, AF.Gelu,
                           "h1", h1p, three_pass=True)
            # split h1 into bf16 hi/lo for the 3-pass second layer
            h1_hi, h1_lo = [], []
            for k in range(KT):
                hh = h1p.tile([128, N], BF16, tag=f"h1h_{k}")
                nc.scalar.copy(hh[:], h1[k][:])
                hl = h1p.tile([128, N], BF16, tag=f"h1l_{k}")
                nc.vector.tensor_tensor(hl[:], h1[k][:], hh[:], op=ALU.subtract)
                h1_hi.append(hh)
                h1_lo.append(hl)
            loff = mlp_layer2(woff2_hi, woff2_lo, h1_hi, h1_lo, ball, 8, 2,
                              AF.Tanh, "loff")
            lxT, lyT = loff

            # ---------- stage B: attention MLP (1-pass bf16) ----------
            xabig = load_xpk(xattpk, KT * N, "xabig")
            xatt_t = [xabig[:, k * N:(k + 1) * N] for k in range(KT)]
            g1 = mlp_layer(Wa1p, [xatt_t], ball, 10, KT, AF.Gelu, "g1", h1p,
                           out_dt=BF16)
            attT = mlp_layer2(watt2, None, g1, None, ball, 18, 1, AF.Copy,
                              "attT", use_dve_bias=True)[0]

            # ---------- stage C: PE transposes to [n, hp] ----------
            def transpose_128x256(src, tag):
                halves = []
                for i in range(2):
                    pt = tps.tile([128, 128], F32, tag="trps")
                    nc.tensor.transpose(pt[:], src[:, i * 128:(i + 1) * 128],
                                        ident[:])
                    o = scr.tile([128, 128], F32, tag=f"{tag}_{i}")
                    nc.vector.tensor_copy(o[:], pt[:])
                    halves.append(o)
                return halves

            lx_n = transpose_128x256(lxT, "lxn")   # [n-tile][128, 128hp]
            ly_n = transpose_128x256(lyT, "lyn")
            att_n = transpose_128x256(attT, "attn")

            # ---------- stage D1: even-base chunk indices in [hp, n] ----------
            # shared scratch slots (sequential chains reuse the same buffers;
            # within a chain each simultaneously-live value has its own slot)
            _sc = [0]

            def s256(i):
                _sc[0] += 1
                return scr.tile([128, N], F32, tag=f"s256_{i}",
                                name=f"s256_{i}_{_sc[0]}")

            def si256():
                _sc[0] += 1
                return scr.tile([128, N], I32, tag="si256",
                                name=f"si256_{_sc[0]}")

            def pos_chain_T(lt, tag):
                gp_ = s256(0)
                nc.vector.tensor_scalar(gp_[:], lt[:], 31.5, 31.5,
                                        op0=ALU.mult, op1=ALU.add)
                nc.vector.tensor_scalar(gp_[:], gp_[:], 62.9999, 0.0,
                                        op0=ALU.min, op1=ALU.max)
                xi = si256()
                nc.vector.tensor_copy(xi[:], gp_[:])
                xf = scr.tile([128, N], F32, tag=f"{tag}_f")
                nc.vector.tensor_copy(xf[:], xi[:])
                wr = s256(1)
                nc.vector.tensor_tensor(wr[:], gp_[:], xf[:], op=ALU.subtract)
                mneg = s256(2)
                nc.vector.tensor_scalar(mneg[:], wr[:], 0.0, None, op0=ALU.is_lt)
                nc.vector.tensor_tensor(xf[:], xf[:], mneg[:], op=ALU.subtract)
                return xf

            xfT = pos_chain_T(lxT, "pxT")
            yfT = pos_chain_T(lyT, "pyT")
            e0f = scr.tile([128, N], F32, tag="e0f")
            nc.vector.scalar_tensor_tensor(e0f[:], yfT[:], 64.0, xfT[:],
                                           op0=ALU.mult, op1=ALU.add)

            ii = scr.tile([128, N], I16, tag="idxi")
            nc.vector.tensor_copy(ii[:], e0f[:])
            # free transpose: Sg[hp, q*16+m] = ii[hp, m*16+q]
            sg = scr.tile([128, N], I16, tag="sg")
            nc.vector.tensor_copy(
                sg[:], _sap(ii, 0, [[1, 16], [16, 16]]))
            # write to idxC[q, h, p, m]: one DMA per q (3-dim AP cap)
            for q in range(16):
                dst = bass.AP(idxC.ap().tensor, q * 2048,
                              [[128, 16], [16, 8], [1, 16]])
                w = nc.sync.dma_start(dst, sg[:, q * 16:(q + 1) * 16])
                cwrite_insts.append(w)

            # ---------- stage D2: 4-slot corner weights t4 in [n, hp] ----------
            # t4[n, col = h*64 + cy*32 + p*4 + j] (bf16), j = x-slot in chunk
            t4_tiles = []

            def s128(i):
                _sc[0] += 1
                return scr.tile([128, 128], F32, tag=f"s128_{i}",
                                name=f"s128_{i}_{_sc[0]}")

            def si128():
                _sc[0] += 1
                return scr.tile([128, 128], I32, tag="si128",
                                name=f"si128_{_sc[0]}")

            for nh in range(2):
                def frac_chain(src_t, tag):
                    g_ = s128(0)
                    nc.vector.tensor_scalar(g_[:], src_t[:], 31.5, 31.5,
                                            op0=ALU.mult, op1=ALU.add)
                    nc.vector.tensor_scalar(g_[:], g_[:], 62.9999, 0.0,
                                            op0=ALU.min, op1=ALU.max)
                    i_ = si128()
                    nc.vector.tensor_copy(i_[:], g_[:])
                    f_ = s128(1)
                    nc.vector.tensor_copy(f_[:], i_[:])
                    wr_ = s128(2)
                    nc.vector.tensor_tensor(wr_[:], g_[:], f_[:], op=ALU.subtract)
                    mn_ = s128(3)
                    nc.vector.tensor_scalar(mn_[:], wr_[:], 0.0, None, op0=ALU.is_lt)
                    w_ = scr.tile([128, 128], F32, tag=f"{tag}_w")
                    nc.vector.tensor_tensor(w_[:], wr_[:], mn_[:], op=ALU.add)
                    fc = scr.tile([128, 128], F32, tag=f"{tag}_fc")
                    nc.vector.tensor_tensor(fc[:], f_[:], mn_[:], op=ALU.subtract)
                    return w_, fc
                wx, _ = frac_chain(lx_n[nh], "fx")
                wy, _ = frac_chain(ly_n[nh], "fy")

                # softmax over p (groups of 8 along free)
                an = att_n[nh]
                mx = scr.tile([128, 16], F32, tag="mx")
                nc.vector.tensor_reduce(
                    mx[:], _sap(an, 0, [[8, 16], [1, 8]]),
                    axis=AX.X, op=ALU.max)
                ex = s128(2)
                nc.vector.tensor_tensor(
                    _sap(ex, 0, [[8, 16], [1, 8]]),
                    _sap(an, 0, [[8, 16], [1, 8]]),
                    _sap(mx, 0, [[1, 16], [0, 8]]),
                    op=ALU.subtract)
                nc.scalar.activation(ex[:], ex[:], AF.Exp)
                sm = scr.tile([128, 16], F32, tag="sm")
                nc.vector.tensor_reduce(
                    sm[:], _sap(ex, 0, [[8, 16], [1, 8]]),
                    axis=AX.X, op=ALU.add)
                rs = scr.tile([128, 16], F32, tag="rs")
                nc.vector.reciprocal(rs[:], sm[:])
                aw = scr.tile([128, 128], F32, tag="aw")
                nc.vector.tensor_tensor(
                    _sap(aw, 0, [[8, 16], [1, 8]]),
                    _sap(ex, 0, [[8, 16], [1, 8]]),
                    _sap(rs, 0, [[1, 16], [0, 8]]),
                    op=ALU.mult)

                # u0 = aw*(1-wx) = aw - aw*wx ; u1 = aw*wx
                u1 = scr.tile([128, 128], F32, tag="u1")
                nc.vector.tensor_tensor(u1[:], aw[:], wx[:], op=ALU.mult)
                u0 = scr.tile([128, 128], F32, tag="u0")
                nc.vector.tensor_tensor(u0[:], aw[:], u1[:], op=ALU.subtract)
                cw1 = wy
                cw0 = scr.tile([128, 128], F32, tag="cw0")
                nc.vector.tensor_scalar(cw0[:], wy[:], -1.0, 1.0,
                                        op0=ALU.mult, op1=ALU.add)

                # t[n, col = h*32 + p*4 + j], j = corner (y,x) in
                # {(0,0),(0,1),(1,0),(1,1)} matching the quad slot order
                tt = scr.tile([128, 512], F32, tag=f"tt_{nh}")
                for j, u, cw in ((0, u0, cw0), (1, u1, cw0),
                                 (2, u0, cw1), (3, u1, cw1)):
                    nc.vector.tensor_tensor(
                        _sap(tt, j, [[32, 16], [4, 8]]),
                        _sap(u, 0, [[8, 16], [1, 8]]),
                        _sap(cw, 0, [[8, 16], [1, 8]]),
                        op=ALU.mult)
                t4_tiles.append(tt)

            # ---------- stage E: v matmul, head-half outer so heads 0-7
            # finish at ~50% and their gathers overlap the second half ----------
            vwrite_half = [[], []]
            for h2 in range(2) if stages >= 2 else []:
                for sup in range(N_SUP):
                    c0 = sup * CTX_SUP
                    big = ctxp.tile([128, KT * CTX_SUP], BF16, tag="chbig",
                                    name=f"chbig_{h2}_{sup}")
                    nc.sync.dma_start(big[:],
                                      ctxpk[sup * 128:(sup + 1) * 128, :])
                    chi = [big[:, k * CTX_SUP:(k + 1) * CTX_SUP]
                           for k in range(KT)]
                    for mm in range(M_PER_SUP):
                        msl = slice(mm * 128, (mm + 1) * 128)
                        vsb = vsbp.tile([128, 512], BF16, tag="vsb",
                                        name=f"vsb_{h2}_{sup}_{mm}")
                        ps = vps.tile([128, 512], F32, tag="vps",
                                      name=f"vps_{h2}_{sup}_{mm}")
                        for k in range(KT):
                            nc.tensor.matmul(
                                ps[:], chi[k][:, msl],
                                wv[k][:, h2 * 512:(h2 + 1) * 512],
                                start=(k == 0), stop=(k == KT - 1))
                        nc.scalar.copy(vsb[:], ps[:])
                        r0 = c0 + mm * 128
                        # rows r hold v[r0+r]; write the 4 footprint slots
                        # (slot r0+r-shift, corner jj), clamping at slot 0
                        for jj, shift, eng in ((0, 0, nc.scalar),
                                               (1, 1, nc.scalar),
                                               (2, 64, nc.sync),
                                               (3, 65, nc.sync)):
                            lo = max(0, shift - r0)
                            dstj = bass.AP(v_dram.ap().tensor,
                                           (r0 + lo - shift) * 4096
                                           + (h2 * 8 + 0) * 256 + jj * DH,
                                           [[4096, 128 - lo], [256, 8],
                                            [1, DH]])
                            w = eng.dma_start(dstj, vsb[lo:128, :])
                            vwrite_half[h2].append(w)
                            vwrite_insts.append(w)

            # ---------- stage F: per-head gather + bf16 reduce ----------
            z = persist.tile([128, 2048], F32, tag="z")  # col = nh*1024 + h*64 + d
            if stages == 4:
                nc.vector.memset(z[:], 0.0)
            all_idx = persist.tile([128, 2048], I16, tag="all_idx")
            for gi_ in range(8) if stages >= 3 or stages == 4 else []:
                src = bass.AP(idxC.ap().tensor, 0, [[2048, 16], [1, 2048]])
                ld = nc.sync.dma_start(all_idx[gi_ * 16:(gi_ + 1) * 16, :], src)
                idxload_insts.append(ld)

            # Batched prepare+trigger gathers: 4 batches x 4 queues so the
            # rings drain concurrently; only batch-0/2 triggers carry the
            # v-write deps (Pool executes triggers in order)
            from concourse.tile_rust import add_dep_helper as _adh
            for b in range(4) if stages >= 3 else []:
                bt, bg, bs = [], [], []
                for h in range(b * 4, b * 4 + 4):
                    g = gp.tile([128, 16, 256], BF16, tag="g", name=f"g_{h}")
                    vsrc = bass.AP(v_dram.ap().tensor, h * 256,
                                   [[4096, CTX], [1, 4 * DH]])
                    gsem = nc.alloc_semaphore(f"gsem_{h}")
                    gi = nc.gpsimd.dma_gather(
                        g[:], vsrc, all_idx[:, h * 128:(h + 1) * 128],
                        2048, 2048, 4 * DH, elem_step=4096,
                        single_packet=False, prepare_only=True, sem=gsem,
                        queue_num=h % 4)
                    gather_insts.append(gi)
                    bg.append(g)
                    bs.append(gsem)
                for qn in range(4):
                    tr = nc.gpsimd.trigger_dma(count=None, queue_num=qn)
                    if b == 0:
                        for w in vwrite_half[0]:
                            _adh(tr.ins, w.ins, reason="gather after vw h0")
                    elif b == 2:
                        for w in vwrite_half[1]:
                            _adh(tr.ins, w.ins, reason="gather after vw h1")
                for hi_, h in enumerate(range(b * 4, b * 4 + 4)):
                    if stages == 4:
                        continue
                    g = bg[hi_]
                    gwait = nc.vector.wait_ge(bs[hi_], 16)
                    for nh in range(2):
                        wt = wtp.tile([128, 2048], BF16, tag="wt")
                        # wt[d*32 + p*4 + j] = G[n, chunk p*2+nh, j*64+d] * t
                        mi = nc.vector.tensor_tensor(
                            _sap(wt, 0, [[32, 64], [4, 8], [1, 4]]),
                            _sap(g, nh * 256, [[1, 64], [512, 8], [64, 4]]),
                            _sap(t4_tiles[nh], h * 32,
                                 [[0, 64], [4, 8], [1, 4]]),
                            op=ALU.mult)
                        _adh(mi.ins, gwait.ins, reason="dve after gather sem")
                        nc.vector.tensor_reduce(
                            _sap(z, nh * 1024 + h * 64, [[1, 64]]),
                            _sap(wt, 0, [[32, 64], [1, 32]]),
                            axis=AX.X, op=ALU.add)

            # ---------- stage G: z -> zT, out projection (1-pass bf16) ----------
            zT = []
            for k in range(KT):
                t = persist.tile([128, N], BF16, tag=f"zT_{k}")
                if stages < 3:
                    nc.vector.memset(t[:], 0.0)
                zT.append(t)
            for nh in range(2) if stages >= 3 else []:
                for c in range(8):
                    pt = tps.tile([128, 128], F32, tag="trps")
                    nc.tensor.transpose(
                        pt[:], z[:, nh * 1024 + c * 128: nh * 1024 + (c + 1) * 128],
                        ident[:])
                    nc.vector.tensor_copy(zT[c][:, nh * 128:(nh + 1) * 128], pt[:])

            for m in range(KT):
                ps = mps.tile([128, N], F32, tag="mlp_ps")
                wt = ws.tile([128, 1024], BF16, tag="wsto", name=f"wst_out_{m}")
                nc.sync.dma_start(wt[:], Woutp[m * 128:(m + 1) * 128, :])
                for k in range(KT):
                    nc.tensor.matmul(ps[:], wt[:, k * 128:(k + 1) * 128], zT[k][:],
                                     start=(k == 0), stop=(k == KT - 1))
                osb = scr.tile([128, N], F32, tag="osb")
                nc.vector.tensor_scalar(osb[:], ps[:], bo[:, m:m + 1], None,
                                        op0=ALU.add)
                nc.sync.dma_start(outT[m * 128:(m + 1) * 128, :], osb[:])

            # ---------- explicit DRAM deps (gather after v/idx writes) ----------
            from concourse.tile_rust import add_dep_helper
            for ld in idxload_insts:
                for w in cwrite_insts:
                    add_dep_helper(ld.ins, w.ins, reason="idx load after idxC write")

    nc.compile()
    return nc


def _prep_inputs(inputs):
    import ml_dtypes
    BFNP = ml_dtypes.bfloat16
    x = np.ascontiguousarray(np.asarray(inputs["x"], dtype=np.float32))
    context = np.asarray(inputs["context"], dtype=np.float32)
    Wv = np.asarray(inputs["Wv"], dtype=np.float32)
    W_off1 = np.asarray(inputs["W_off1"], np.float32)
    W_off2 = np.asarray(inputs["W_off2"], dtype=np.float32)
    b_off2 = np.asarray(inputs["b_off2"], dtype=np.float32)

    def split(a):
        hi = a.astype(BFNP)
        lo = (a - hi.astype(np.float32)).astype(BFNP)
        return hi, lo

    def pack_stationary(W):
        # W [DIM, mt*128] -> out[m*128+p, k*128+j] = W[k*128+p, m*128+j]
        mt = W.shape[1] // 128
        W4 = W.reshape(KT, 128, mt, 128)
        return np.ascontiguousarray(
            W4.transpose(2, 1, 0, 3).reshape(mt * 128, KT * 128))

    def pack_wide(W):
        # W [DIM, C] -> out[p, k*C + c] = W[k*128+p, c]
        C = W.shape[1]
        W3 = W.reshape(KT, 128, C)
        return np.ascontiguousarray(W3.transpose(1, 0, 2).reshape(128, KT * C))

    cy, h, p = np.meshgrid(np.arange(2), np.arange(HEADS), np.arange(P),
                           indexing="ij")
    oldcol = (h * 16 + p * 2 + cy).reshape(-1)
    W_off2p = np.ascontiguousarray(W_off2[:, oldcol])
    b_off2p = np.ascontiguousarray(b_off2[oldcol])

    Wo1_hi, Wo1_lo = split(W_off1)
    # per m-block: [m*128+p, lvl*1024 + k*128+j]
    Wo1p = np.concatenate(
        [np.concatenate([pack_stationary(Wo1_hi.astype(np.float32))[m * 128:(m + 1) * 128],
                         pack_stationary(Wo1_lo.astype(np.float32))[m * 128:(m + 1) * 128]],
                        axis=1) for m in range(KT)], axis=0).astype(BFNP)

    Wo2_hi, Wo2_lo = split(W_off2p)
    Wo2pk = np.concatenate([pack_wide(Wo2_hi.astype(np.float32)),
                            pack_wide(Wo2_lo.astype(np.float32))],
                           axis=1).astype(BFNP)
    Wa1p = pack_stationary(
        np.asarray(inputs["W_att1"], np.float32)).astype(BFNP)
    Wa2pk = pack_wide(np.asarray(inputs["W_att2"], np.float32)).astype(BFNP)
    Woutp = pack_stationary(
        np.asarray(inputs["W_out"], np.float32)).astype(BFNP)

    bpack = np.zeros((128, 27), np.float32)
    bpack[:, 0:8] = np.asarray(inputs["b_off1"], np.float32).reshape(8, 128).T
    bpack[:, 8:10] = b_off2p.reshape(2, 128).T
    bpack[:, 10:18] = np.asarray(inputs["b_att1"], np.float32).reshape(8, 128).T
    bpack[:, 18:19] = np.asarray(inputs["b_att2"], np.float32).reshape(1, 128).T
    bpack[:, 19:27] = np.asarray(inputs["b_out"], np.float32).reshape(8, 128).T

    common = {
        "Wv": np.ascontiguousarray(Wv.astype(BFNP)),
        "Wo1p": np.ascontiguousarray(Wo1p),
        "Wo2pk": np.ascontiguousarray(Wo2pk),
        "Wa1p": np.ascontiguousarray(Wa1p),
        "Wa2pk": np.ascontiguousarray(Wa2pk),
        "Woutp": np.ascontiguousarray(Woutp),
        "bpack": bpack,
    }

    xflat = x.reshape(B * N, DIM)
    n_idx = np.arange(N)
    in_maps = []
    for b in range(B):
        perm = (n_idx // 32) * 256 + (n_idx % 32) * 8 + b
        xoffT = np.ascontiguousarray(xflat[perm].T)
        xo_hi, xo_lo = split(xoffT)
        xoffpk = np.concatenate(
            [pack_wide(xo_hi.astype(np.float32)),
             pack_wide(xo_lo.astype(np.float32))], axis=1).astype(BFNP)
        xattpk = pack_wide(x[b].T).astype(BFNP)
        ctxT = context[b].T.astype(BFNP).astype(np.float32)
        # ctxpk[sup*128+p, k*512+c] = ctxT[k*128+p, sup*512+c]
        C4 = ctxT.reshape(KT, 128, N_SUP, CTX_SUP)
        ctxpk = np.ascontiguousarray(
            C4.transpose(2, 1, 0, 3).reshape(N_SUP * 128,
                                             KT * CTX_SUP)).astype(BFNP)
        m = dict(common)
        m.update({"ctxpk": ctxpk, "xoffpk": np.ascontiguousarray(xoffpk),
                  "xattpk": np.ascontiguousarray(xattpk)})
        in_maps.append(m)
    return in_maps


def kernel(**inputs):
    if "nc" not in _CACHE:
        _CACHE["nc"] = _build()
    nc = _CACHE["nc"]
    in_maps = _prep_inputs(inputs)
    res = run_bass_kernel_spmd(nc, in_maps, list(range(8)))
    _CACHE["last_results"] = res
    out = np.stack([res.results[i]["outT"].T for i in range(B)], axis=0)
    return np.ascontiguousarray(out.astype(np.float32))


# revision 35
# speedup vs baseline: 1.0366x; 1.0366x over previous
"""Trainium2 Bass kernel for nn_DeformableCrossAttention.

Sharding: data-parallel over batch B=8 across 8 NeuronCores (one sample per
core).  Inside each core:
  - offset MLP in 3-pass bf16 hi/lo split (position precision matters:
    output error ~ 1.4x the position error in pixels; 3-pass keeps it ~2e-5)
  - attention MLP + out-projection in 1-pass bf16
  - v = context @ Wv in 1-pass bf16, stored bf16 in DRAM
  - bilinear sampling via per-head SWDGE dma_gather of 512B chunks, each
    covering FOUR spatial positions (even base e=floor(lin/2), parity of the
    true x0 folded into the DVE corner weights t4)
  - attention-weighted bilinear reduce on DVE in bf16 (4 x-slots per corner
    row, zero weight on the two unused slots)
  - output emitted transposed; host transposes back.

All weight/x streams host-packed into contiguous per-m-tile layouts so every
DMA is >=2KB-run contiguous.  Self-contained: hardcodes all problem shapes.
"""
import sys
sys.path.insert(0, "/opt/trn_rl_repo")

import numpy as np
import concourse.bass as bass
import concourse.mybir as mybir
import concourse.tile as tile
from concourse import bacc
from concourse.bass_utils import run_bass_kernel_spmd
from concourse.masks import make_identity

F32 = mybir.dt.float32
BF16 = mybir.dt.bfloat16
I16 = mybir.dt.int16
I32 = mybir.dt.int32
AF = mybir.ActivationFunctionType
ALU = mybir.AluOpType
AX = mybir.AxisListType

B, N, DIM = 8, 256, 1024
HEADS, DH, P = 16, 64, 8
HS = WS = 64
CTX = HS * WS            # 4096
INNER = HEADS * DH       # 1024
KT = DIM // 128          # 8 k-tiles
PLANE = CTX * DH         # per-head v plane elements (262144, bf16)

CTX_SUP = 512            # ctx supertile rows
N_SUP = CTX // CTX_SUP   # 8 supertiles
M_PER_SUP = CTX_SUP // 128

_CACHE = {}


def _ap(t, offset, dims):
    return bass.AP(t.ap().tensor if hasattr(t, "ap") else t.tensor, offset, dims)


def _sap(tile_obj, extra, dims):
    """Sub-AP of an SBUF tile: keep its partition dim, custom free dims,
    extra offset in elements."""
    a = tile_obj[:]
    return bass.AP(a.tensor, a.offset + extra, [list(a.ap[0])] + dims)


def _build(repeat=1, stages=3):
    nc = bacc.Bacc("TRN2", target_bir_lowering=False, debug=False,
                   num_swdge_queues=4)

    # ---------------- I/O (host-packed layouts) ----------------
    # ctxpk[sup*128+p, k*512+c] = bf16(ctxT)[k*128+p, sup*512+c]
    ctxpk = nc.dram_tensor("ctxpk", [N_SUP * 128, KT * CTX_SUP], BF16,
                           kind="ExternalInput")
    Wv = nc.dram_tensor("Wv", [DIM, INNER], BF16, kind="ExternalInput")
    # xoffpk[p, lvl*2048 + k*256 + n] (lvl 0=hi, 1=lo)
    xoffpk = nc.dram_tensor("xoffpk", [128, 2 * KT * N], BF16,
                            kind="ExternalInput")
    xattpk = nc.dram_tensor("xattpk", [128, KT * N], BF16, kind="ExternalInput")
    # Wo1p[m*128+p, lvl*1024 + k*128+j]
    Wo1p = nc.dram_tensor("Wo1p", [KT * 128, 2 * DIM], BF16,
                          kind="ExternalInput")
    # Wo2pk[p, lvl*2048 + k*256 + c]
    Wo2pk = nc.dram_tensor("Wo2pk", [128, 2 * KT * 256], BF16,
                           kind="ExternalInput")
    # Wa1p[m*128+p, k*128+j]
    Wa1p = nc.dram_tensor("Wa1p", [KT * 128, DIM], BF16, kind="ExternalInput")
    # Wa2pk[p, k*128+c]
    Wa2pk = nc.dram_tensor("Wa2pk", [128, KT * 128], BF16, kind="ExternalInput")
    # Woutp[m*128+p, k*128+j]
    Woutp = nc.dram_tensor("Woutp", [KT * 128, DIM], BF16, kind="ExternalInput")
    # bpack[p, col]: 0:8 b_off1, 8:10 b_off2p, 10:18 b_att1, 18 b_att2,
    # 19:27 b_out
    bpack = nc.dram_tensor("bpack", [128, 27], F32, kind="ExternalInput")

    outT = nc.dram_tensor("outT", [DIM, N], F32, kind="ExternalOutput")

    # DRAM scratch: bf16 quad layout [slot, head, 4, DH]: slot s holds the
    # full 2x2 bilinear footprint [v[s], v[s+1], v[s+64], v[s+65]] per head
    # (512B chunks, 8KB stride) so ONE gather descriptor covers all 4 corners
    v_dram = nc.dram_tensor("v_dram", [CTX, HEADS, 4, DH], BF16)
    # idxC[q, h, p, m] int16
    idxC = nc.dram_tensor("idxC", [16, HEADS, P, 16], I16)

    vwrite_insts = []
    cwrite_insts = []
    gather_insts = []
    idxload_insts = []

    with tile.TileContext(nc) as tc:
        import contextlib
        with contextlib.ExitStack() as ctx:
            persist = ctx.enter_context(tc.tile_pool(name="persist", bufs=1))
            ws = ctx.enter_context(tc.tile_pool(name="wstream", bufs=2))
            h1p = ctx.enter_context(tc.tile_pool(name="h1p", bufs=1))
            ctxp = ctx.enter_context(tc.tile_pool(name="ctxp", bufs=2))
            vsbp = ctx.enter_context(tc.tile_pool(name="vsbp", bufs=2))
            gp = ctx.enter_context(tc.tile_pool(name="gp", bufs=6))
            wtp = ctx.enter_context(tc.tile_pool(name="wtp", bufs=2))
            scr = ctx.enter_context(tc.tile_pool(name="scr", bufs=1))
            mps = ctx.enter_context(tc.tile_pool(name="mps", bufs=2, space="PSUM"))
            vps = ctx.enter_context(tc.tile_pool(name="vps", bufs=4, space="PSUM"))
            tps = ctx.enter_context(tc.tile_pool(name="tps", bufs=2, space="PSUM"))
            if repeat > 1:
                ctx.enter_context(tc.For_i(0, repeat, 1))

            # ---------- persistent loads ----------
            def load_tiles(dram, rows, cols, dt, tag):
                ts_ = []
                for k in range(rows // 128):
                    t = persist.tile([128, cols], dt, tag=f"{tag}_{k}")
                    nc.sync.dma_start(t[:], dram[k * 128:(k + 1) * 128, :])
                    ts_.append(t)
                return ts_

            wv = load_tiles(Wv, DIM, INNER, BF16, "wv")

            def load_packed(dram, cols, tag):
                big = persist.tile([128, cols], BF16, tag=tag, name=tag)
                nc.sync.dma_start(big[:], dram[:, :])
                return big

            wo2big = load_packed(Wo2pk, 2 * KT * 256, "wo2big")
            woff2_hi = [wo2big[:, k * 256:(k + 1) * 256] for k in range(KT)]
            woff2_lo = [wo2big[:, 2048 + k * 256:2048 + (k + 1) * 256]
                        for k in range(KT)]
            wa2big = load_packed(Wa2pk, KT * 128, "wa2big")
            watt2 = [wa2big[:, k * 128:(k + 1) * 128] for k in range(KT)]

            ball = persist.tile([128, 27], F32, tag="ball")
            nc.sync.dma_start(ball[:], bpack[:, :])
            bo1 = ball[:, 0:8]
            bo2 = ball[:, 8:10]
            ba1 = ball[:, 10:18]
            ba2 = ball[:, 18:19]
            bo = ball[:, 19:27]

            ident = persist.tile([128, 128], F32, tag="ident")
            make_identity(nc, ident[:])

            # ---------- MLP layer 1: stream packed W, psum-accumulate ----------
            def mlp_layer(w_dram, x_tile_lists, bias_tile, bcol0, mtiles, act,
                          out_tag, pool, out_dt=F32, three_pass=False):
                wcols = 2 * DIM if three_pass else DIM
                outs = []
                for m in range(mtiles):
                    wt = ws.tile([128, wcols], BF16, tag="wst",
                                 name=f"wst_{out_tag}_{m}")
                    nc.sync.dma_start(wt[:], w_dram[m * 128:(m + 1) * 128, :])
                    if three_pass:
                        whi = [wt[:, k * 128:(k + 1) * 128] for k in range(KT)]
                        wlo = [wt[:, DIM + k * 128:DIM + (k + 1) * 128]
                               for k in range(KT)]
                        passes = [(whi, x_tile_lists[0]),
                                  (whi, x_tile_lists[1]),
                                  (wlo, x_tile_lists[0])]
                    else:
                        whi = [wt[:, k * 128:(k + 1) * 128] for k in range(KT)]
                        passes = [(whi, x_tile_lists[0])]
                    ps = mps.tile([128, N], F32, tag="mlp_ps")
                    np_ = len(passes)
                    for pi, (wl, xl) in enumerate(passes):
                        for k in range(KT):
                            nc.tensor.matmul(ps[:], wl[k], xl[k],
                                             start=(pi == 0 and k == 0),
                                             stop=(pi == np_ - 1 and k == KT - 1))
                    o = pool.tile([128, N], out_dt, tag=f"{out_tag}_{m}")
                    nc.scalar.activation(o[:], ps[:], act,
                                         bias=bias_tile[:, bcol0 + m:bcol0 + m + 1])
                    outs.append(o)
                return outs

            def mlp_layer2(w_hi, w_lo, x_hi, x_lo, bias_tile, bcol0, mtiles,
                           act, out_tag, use_dve_bias=False):
                if w_lo is not None:
                    passes = [(w_hi, x_hi), (w_lo, x_hi), (w_hi, x_lo)]
                else:
                    passes = [(w_hi, x_hi)]
                outs = []
                for m in range(mtiles):
                    ps = mps.tile([128, N], F32, tag="mlp_ps")
                    np_ = len(passes)
                    for pi, (wl, xl) in enumerate(passes):
                        for k in range(KT):
                            nc.tensor.matmul(
                                ps[:], wl[k][:, m * 128:(m + 1) * 128], xl[k][:],
                                start=(pi == 0 and k == 0),
                                stop=(pi == np_ - 1 and k == KT - 1))
                    o = scr.tile([128, N], F32, tag=f"{out_tag}_{m}")
                    if use_dve_bias:
                        nc.vector.tensor_scalar(
                            o[:], ps[:], bias_tile[:, bcol0 + m:bcol0 + m + 1],
                            None, op0=ALU.add)
                    else:
                        nc.scalar.activation(
                            o[:], ps[:], act,
                            bias=bias_tile[:, bcol0 + m:bcol0 + m + 1])
                    outs.append(o)
                return outs

            # ---------- stage A: offset MLP (3-pass bf16) ----------
            def load_xpk(dram, cols, name):
                big = h1p.tile([128, cols], BF16, tag=name, name=name)
                nc.sync.dma_start(big[:], dram[:, :])
                return big

            xobig = load_xpk(xoffpk, 2 * KT * N, "xobig")
            xoff_hi = [xobig[:, k * N:(k + 1) * N] for k in range(KT)]
            xoff_lo = [xobig[:, KT * N + k * N:KT * N + (k + 1) * N]
                       for k in range(KT)]
            h1 = mlp_layer(Wo1p, [xoff_hi, xoff_lo], ball, 0, KT, AF.Gelu,
                           "h1", h1p, three_pass=True)
            # split h1 into bf16 hi/lo for the 3-pass second layer
            h1_hi, h1_lo = [], []
            for k in range(KT):
                hh = h1p.tile([128, N], BF16, tag=f"h1h_{k}")
                nc.scalar.copy(hh[:], h1[k][:])
                hl = h1p.tile([128, N], BF16, tag=f"h1l_{k}")
                nc.vector.tensor_tensor(hl[:], h1[k][:], hh[:], op=ALU.subtract)
                h1_hi.append(hh)
                h1_lo.append(hl)
            loff = mlp_layer2(woff2_hi, woff2_lo, h1_hi, h1_lo, ball, 8, 2,
                              AF.Tanh, "loff")
            lxT, lyT = loff

            # ---------- stage B: attention MLP (1-pass bf16) ----------
            xabig = load_xpk(xattpk, KT * N, "xabig")
            xatt_t = [xabig[:, k * N:(k + 1) * N] for k in range(KT)]
            g1 = mlp_layer(Wa1p, [xatt_t], ball, 10, KT, AF.Gelu, "g1", h1p,
                           out_dt=BF16)
            attT = mlp_layer2(watt2, None, g1, None, ball, 18, 1, AF.Copy,
                              "attT", use_dve_bias=True)[0]

            # ---------- stage C: PE transposes to [n, hp] ----------
            def transpose_128x256(src, tag):
                halves = []
                for i in range(2):
                    pt = tps.tile([128, 128], F32, tag="trps")
                    nc.tensor.transpose(pt[:], src[:, i * 128:(i + 1) * 128],
                                        ident[:])
                    o = scr.tile([128, 128], F32, tag=f"{tag}_{i}")
                    nc.vector.tensor_copy(o[:], pt[:])
                    halves.append(o)
                return halves

            lx_n = transpose_128x256(lxT, "lxn")   # [n-tile][128, 128hp]
            ly_n = transpose_128x256(lyT, "lyn")
            att_n = transpose_128x256(attT, "attn")

            # ---------- stage D1: even-base chunk indices in [hp, n] ----------
            # shared scratch slots (sequential chains reuse the same buffers;
            # within a chain each simultaneously-live value has its own slot)
            _sc = [0]

            def s256(i):
                _sc[0] += 1
                return scr.tile([128, N], F32, tag=f"s256_{i}",
                                name=f"s256_{i}_{_sc[0]}")

            def si256():
                _sc[0] += 1
                return scr.tile([128, N], I32, tag="si256",
                                name=f"si256_{_sc[0]}")

            def pos_chain_T(lt, tag):
                gp_ = s256(0)
                nc.vector.tensor_scalar(gp_[:], lt[:], 31.5, 31.5,
                                        op0=ALU.mult, op1=ALU.add)
                nc.vector.tensor_scalar(gp_[:], gp_[:], 62.9999, 0.0,
                                        op0=ALU.min, op1=ALU.max)
                xi = si256()
                nc.vector.tensor_copy(xi[:], gp_[:])
                xf = scr.tile([128, N], F32, tag=f"{tag}_f")
                nc.vector.tensor_copy(xf[:], xi[:])
                wr = s256(1)
                nc.vector.tensor_tensor(wr[:], gp_[:], xf[:], op=ALU.subtract)
                mneg = s256(2)
                nc.vector.tensor_scalar(mneg[:], wr[:], 0.0, None, op0=ALU.is_lt)
                nc.vector.tensor_tensor(xf[:], xf[:], mneg[:], op=ALU.subtract)
                return xf

            xfT = pos_chain_T(lxT, "pxT")
            yfT = pos_chain_T(lyT, "pyT")
            e0f = scr.tile([128, N], F32, tag="e0f")
            nc.vector.scalar_tensor_tensor(e0f[:], yfT[:], 64.0, xfT[:],
                                           op0=ALU.mult, op1=ALU.add)

            ii = scr.tile([128, N], I16, tag="idxi")
            nc.vector.tensor_copy(ii[:], e0f[:])
            # free transpose: Sg[hp, q*16+m] = ii[hp, m*16+q]
            sg = scr.tile([128, N], I16, tag="sg")
            nc.vector.tensor_copy(
                sg[:], _sap(ii, 0, [[1, 16], [16, 16]]))
            # write to idxC[q, h, p, m]: one DMA per q (3-dim AP cap)
            for q in range(16):
                dst = bass.AP(idxC.ap().tensor, q * 2048,
                              [[128, 16], [16, 8], [1, 16]])
                w = nc.sync.dma_start(dst, sg[:, q * 16:(q + 1) * 16])
                cwrite_insts.append(w)

            # ---------- stage D2: 4-slot corner weights t4 in [n, hp] ----------
            # t4[n, col = h*64 + cy*32 + p*4 + j] (bf16), j = x-slot in chunk
            t4_tiles = []

            def s128(i):
                _sc[0] += 1
                return scr.tile([128, 128], F32, tag=f"s128_{i}",
                                name=f"s128_{i}_{_sc[0]}")

            def si128():
                _sc[0] += 1
                return scr.tile([128, 128], I32, tag="si128",
                                name=f"si128_{_sc[0]}")

            for nh in range(2):
                def frac_chain(src_t, tag):
                    g_ = s128(0)
                    nc.vector.tensor_scalar(g_[:], src_t[:], 31.5, 31.5,
                                            op0=ALU.mult, op1=ALU.add)
                    nc.vector.tensor_scalar(g_[:], g_[:], 62.9999, 0.0,
                                            op0=ALU.min, op1=ALU.max)
                    i_ = si128()
                    nc.vector.tensor_copy(i_[:], g_[:])
                    f_ = s128(1)
                    nc.vector.tensor_copy(f_[:], i_[:])
                    wr_ = s128(2)
                    nc.vector.tensor_tensor(wr_[:], g_[:], f_[:], op=ALU.subtract)
                    mn_ = s128(3)
                    nc.vector.tensor_scalar(mn_[:], wr_[:], 0.0, None, op0=ALU.is_lt)
                    w_ = scr.tile([128, 128], F32, tag=f"{tag}_w")
                    nc.vector.tensor_tensor(w_[:], wr_[:], mn_[:], op=ALU.add)
                    fc = scr.tile([128, 128], F32, tag=f"{tag}_fc")
                    nc.vector.tensor_tensor(fc[:], f_[:], mn_[:], op=ALU.subtract)
                    return w_, fc
                wx, _ = frac_chain(lx_n[nh], "fx")
                wy, _ = frac_chain(ly_n[nh], "fy")

                # softmax over p (groups of 8 along free)
                an = att_n[nh]
                mx = scr.tile([128, 16], F32, tag="mx")
                nc.vector.tensor_reduce(
                    mx[:], _sap(an, 0, [[8, 16], [1, 8]]),
                    axis=AX.X, op=ALU.max)
                ex = s128(2)
                nc.vector.tensor_tensor(
                    _sap(ex, 0, [[8, 16], [1, 8]]),
                    _sap(an, 0, [[8, 16], [1, 8]]),
                    _sap(mx, 0, [[1, 16], [0, 8]]),
                    op=ALU.subtract)
                nc.scalar.activation(ex[:], ex[:], AF.Exp)
                sm = scr.tile([128, 16], F32, tag="sm")
                nc.vector.tensor_reduce(
                    sm[:], _sap(ex, 0, [[8, 16], [1, 8]]),
                    axis=AX.X, op=ALU.add)
                rs = scr.tile([128, 16], F32, tag="rs")
                nc.vector.reciprocal(rs[:], sm[:])
                aw = scr.tile([128, 128], F32, tag="aw")
                nc.vector.tensor_tensor(
                    _sap(aw, 0, [[8, 16], [1, 8]]),
                    _sap(ex, 0, [[8, 16], [1, 8]]),
                    _sap(rs, 0, [[1, 16], [0, 8]]),
                    op=ALU.mult)

                # u0 = aw*(1-wx) = aw - aw*wx ; u1 = aw*wx
                u1 = scr.tile([128, 128], F32, tag="u1")
                nc.vector.tensor_tensor(u1[:], aw[:], wx[:], op=ALU.mult)
                u0 = scr.tile([128, 128], F32, tag="u0")
                nc.vector.tensor_tensor(u0[:], aw[:], u1[:], op=ALU.subtract)
                cw1 = wy
                cw0 = scr.tile([128, 128], F32, tag="cw0")
                nc.vector.tensor_scalar(cw0[:], wy[:], -1.0, 1.0,
                                        op0=ALU.mult, op1=ALU.add)

                # t[n, col = h*32 + p*4 + j], j = corner (y,x) in
                # {(0,0),(0,1),(1,0),(1,1)} matching the quad slot order
                tt = scr.tile([128, 512], F32, tag=f"tt_{nh}")
                for j, u, cw in ((0, u0, cw0), (1, u1, cw0),
                                 (2, u0, cw1), (3, u1, cw1)):
                    nc.vector.tensor_tensor(
                        _sap(tt, j, [[32, 16], [4, 8]]),
                        _sap(u, 0, [[8, 16], [1, 8]]),
                        _sap(cw, 0, [[8, 16], [1, 8]]),
                        op=ALU.mult)
                t4_tiles.append(tt)

            # ---------- stage E: v matmul, head-half outer so heads 0-7
            # finish at ~50% and their gathers overlap the second half ----------
            vwrite_half = [[], []]
            for h2 in range(2) if stages >= 2 else []:
                for sup in range(N_SUP):
                    c0 = sup * CTX_SUP
                    big = ctxp.tile([128, KT * CTX_SUP], BF16, tag="chbig",
                                    name=f"chbig_{h2}_{sup}")
                    nc.sync.dma_start(big[:],
                                      ctxpk[sup * 128:(sup + 1) * 128, :])
                    chi = [big[:, k * CTX_SUP:(k + 1) * CTX_SUP]
                           for k in range(KT)]
                    for mm in range(M_PER_SUP):
                        msl = slice(mm * 128, (mm + 1) * 128)
                        vsb = vsbp.tile([128, 512], BF16, tag="vsb",
                                        name=f"vsb_{h2}_{sup}_{mm}")
                        ps = vps.tile([128, 512], F32, tag="vps",
                                      name=f"vps_{h2}_{sup}_{mm}")
                        for k in range(KT):
                            nc.tensor.matmul(
                                ps[:], chi[k][:, msl],
                                wv[k][:, h2 * 512:(h2 + 1) * 512],
                                start=(k == 0), stop=(k == KT - 1))
                        nc.scalar.copy(vsb[:], ps[:])
                        r0 = c0 + mm * 128
                        # rows r hold v[r0+r]; write the 4 footprint slots
                        # (slot r0+r-shift, corner jj), clamping at slot 0
                        for jj, shift, eng in ((0, 0, nc.scalar),
                                               (1, 1, nc.scalar),
                                               (2, 64, nc.sync),
                                               (3, 65, nc.sync)):
                            lo = max(0, shift - r0)
                            dstj = bass.AP(v_dram.ap().tensor,
                                           (r0 + lo - shift) * 4096
                                           + (h2 * 8 + 0) * 256 + jj * DH,
                                           [[4096, 128 - lo], [256, 8],
                                            [1, DH]])
                            w = eng.dma_start(dstj, vsb[lo:128, :])
                            vwrite_half[h2].append(w)
                            vwrite_insts.append(w)

            # ---------- stage F: per-head gather + bf16 reduce ----------
            z = persist.tile([128, 2048], F32, tag="z")  # col = nh*1024 + h*64 + d
            if stages == 4:
                nc.vector.memset(z[:], 0.0)
            all_idx = persist.tile([128, 2048], I16, tag="all_idx")
            for gi_ in range(8) if stages >= 3 or stages == 4 else []:
                src = bass.AP(idxC.ap().tensor, 0, [[2048, 16], [1, 2048]])
                ld = nc.sync.dma_start(all_idx[gi_ * 16:(gi_ + 1) * 16, :], src)
                idxload_insts.append(ld)

            # Batched prepare+trigger gathers: 4 batches x 4 queues so the
            # rings drain concurrently; only batch-0/2 triggers carry the
            # v-write deps (Pool executes triggers in order)
            from concourse.tile_rust import add_dep_helper as _adh
            for b in range(4) if stages >= 3 else []:
                bt, bg, bs = [], [], []
                for h in range(b * 4, b * 4 + 4):
                    g = gp.tile([128, 16, 256], BF16, tag="g", name=f"g_{h}")
                    vsrc = bass.AP(v_dram.ap().tensor, h * 256,
                                   [[4096, CTX], [1, 4 * DH]])
                    gsem = nc.alloc_semaphore(f"gsem_{h}")
                    gi = nc.gpsimd.dma_gather(
                        g[:], vsrc, all_idx[:, h * 128:(h + 1) * 128],
                        2048, 2048, 4 * DH, elem_step=4096,
                        single_packet=False, prepare_only=True, sem=gsem,
                        queue_num=h % 4)
                    gather_insts.append(gi)
                    bg.append(g)
                    bs.append(gsem)
                for qn in range(4):
                    tr = nc.gpsimd.trigger_dma(count=None, queue_num=qn)
                    if b == 0:
                        for w in vwrite_half[0]:
                            _adh(tr.ins, w.ins, reason="gather after vw h0")
                    elif b == 2:
                        for w in vwrite_half[1]:
                            _adh(tr.ins, w.ins, reason="gather after vw h1")
                for hi_, h in enumerate(range(b * 4, b * 4 + 4)):
                    if stages == 4:
                        continue
                    g = bg[hi_]
                    gwait = nc.vector.wait_ge(bs[hi_], 16)
                    for nh in range(2):
                        wt = wtp.tile([128, 2048], F32, tag="wt")
                        # wt[d*32 + p*4 + j] = G[n, chunk p*2+nh, j*64+d] * t
                        mi = nc.vector.tensor_tensor(
                            _sap(wt, 0, [[32, 64], [4, 8], [1, 4]]),
                            _sap(g, nh * 256, [[1, 64], [512, 8], [64, 4]]),
                            _sap(t4_tiles[nh], h * 32,
                                 [[0, 64], [4, 8], [1, 4]]),
                            op=ALU.mult)
                        _adh(mi.ins, gwait.ins, reason="dve after gather sem")
                        nc.vector.tensor_reduce(
                            _sap(z, nh * 1024 + h * 64, [[1, 64]]),
                            _sap(wt, 0, [[32, 64], [1, 32]]),
                            axis=AX.X, op=ALU.add)

            # ---------- stage G: z -> zT, out projection (1-pass bf16) ----------
            zT = []
            for k in range(KT):
                t = persist.tile([128, N], BF16, tag=f"zT_{k}")
                if stages < 3:
                    nc.vector.memset(t[:], 0.0)
                zT.append(t)
            for nh in range(2) if stages >= 3 else []:
                for c in range(8):
                    pt = tps.tile([128, 128], F32, tag="trps")
                    nc.tensor.transpose(
                        pt[:], z[:, nh * 1024 + c * 128: nh * 1024 + (c + 1) * 128],
                        ident[:])
                    nc.vector.tensor_copy(zT[c][:, nh * 128:(nh + 1) * 128], pt[:])

            for m in range(KT):
                ps = mps.tile([128, N], F32, tag="mlp_ps")
                wt = ws.tile([128, 1024], BF16, tag="wsto", name=f"wst_out_{m}")
                nc.sync.dma_start(wt[:], Woutp[m * 128:(m + 1) * 128, :])
                for k in range(KT):
                    nc.tensor.matmul(ps[:], wt[:, k * 128:(k + 1) * 128], zT[k][:],
                                     start=(k == 0), stop=(k == KT - 1))
                osb = scr.tile([128, N], F32, tag="osb")
                nc.vector.tensor_scalar(osb[:], ps[:], bo[:, m:m + 1], None,
                                        op0=ALU.add)
                nc.sync.dma_start(outT[m * 128:(m + 1) * 128, :], osb[:])

            # ---------- explicit DRAM deps (gather after v/idx writes) ----------
            from concourse.tile_rust import add_dep_helper
            for ld in idxload_insts:
                for w in cwrite_insts:
                    add_dep_helper(ld.ins, w.ins, reason="idx load after idxC write")

    nc.compile()
    return nc


def _prep_inputs(inputs):
    import ml_dtypes
    BFNP = ml_dtypes.bfloat16
    x = np.ascontiguousarray(np.asarray(inputs["x"], dtype=np.float32))
    context = np.asarray(inputs["context"], dtype=np.float32)
    Wv = np.asarray(inputs["Wv"], dtype=np.float32)
    W_off1 = np.asarray(inputs["W_off1"], np.float32)
    W_off2 = np.asarray(inputs["W_off2"], dtype=np.float32)
    b_off2 = np.asarray(inputs["b_off2"], dtype=np.float32)

    def split(a):
        hi = a.astype(BFNP)
        lo = (a - hi.astype(np.float32)).astype(BFNP)
        return hi, lo

    def pack_stationary(W):
        # W [DIM, mt*128] -> out[m*128+p, k*128+j] = W[k*128+p, m*128+j]
        mt = W.shape[1] // 128
        W4 = W.reshape(KT, 128, mt, 128)
        return np.ascontiguousarray(
            W4.transpose(2, 1, 0, 3).reshape(mt * 128, KT * 128))

    def pack_wide(W):
        # W [DIM, C] -> out[p, k*C + c] = W[k*128+p, c]
        C = W.shape[1]
        W3 = W.reshape(KT, 128, C)
        return np.ascontiguousarray(W3.transpose(1, 0, 2).reshape(128, KT * C))

    cy, h, p = np.meshgrid(np.arange(2), np.arange(HEADS), np.arange(P),
                           indexing="ij")
    oldcol = (h * 16 + p * 2 + cy).reshape(-1)
    W_off2p = np.ascontiguousarray(W_off2[:, oldcol])
    b_off2p = np.ascontiguousarray(b_off2[oldcol])

    Wo1_hi, Wo1_lo = split(W_off1)
    # per m-block: [m*128+p, lvl*1024 + k*128+j]
    Wo1p = np.concatenate(
        [np.concatenate([pack_stationary(Wo1_hi.astype(np.float32))[m * 128:(m + 1) * 128],
                         pack_stationary(Wo1_lo.astype(np.float32))[m * 128:(m + 1) * 128]],
                        axis=1) for m in range(KT)], axis=0).astype(BFNP)

    Wo2_hi, Wo2_lo = split(W_off2p)
    Wo2pk = np.concatenate([pack_wide(Wo2_hi.astype(np.float32)),
                            pack_wide(Wo2_lo.astype(np.float32))],
                           axis=1).astype(BFNP)
    Wa1p = pack_stationary(
        np.asarray(inputs["W_att1"], np.float32)).astype(BFNP)
    Wa2pk = pack_wide(np.asarray(inputs["W_att2"], np.float32)).astype(BFNP)
    Woutp = pack_stationary(
        np.asarray(inputs["W_out"], np.float32)).astype(BFNP)

    bpack = np.zeros((128, 27), np.float32)
    bpack[:, 0:8] = np.asarray(inputs["b_off1"], np.float32).reshape(8, 128).T
    bpack[:, 8:10] = b_off2p.reshape(2, 128).T
    bpack[:, 10:18] = np.asarray(inputs["b_att1"], np.float32).reshape(8, 128).T
    bpack[:, 18:19] = np.asarray(inputs["b_att2"], np.float32).reshape(1, 128).T
    bpack[:, 19:27] = np.asarray(inputs["b_out"], np.float32).reshape(8, 128).T

    common = {
        "Wv": np.ascontiguousarray(Wv.astype(BFNP)),
        "Wo1p": np.ascontiguousarray(Wo1p),
        "Wo2pk": np.ascontiguousarray(Wo2pk),
        "Wa1p": np.ascontiguousarray(Wa1p),
        "Wa2pk": np.ascontiguousarray(Wa2pk),
        "Woutp": np.ascontiguousarray(Woutp),
        "bpack": bpack,
    }

    xflat = x.reshape(B * N, DIM)
    n_idx = np.arange(N)
    in_maps = []
    for b in range(B):
        perm = (n_idx // 32) * 256 + (n_idx % 32) * 8 + b
        xoffT = np.ascontiguousarray(xflat[perm].T)
        xo_hi, xo_lo = split(xoffT)
        xoffpk = np.concatenate(
            [pack_wide(xo_hi.astype(np.float32)),
             pack_wide(xo_lo.astype(np.float32))], axis=1).astype(BFNP)
        xattpk = pack_wide(x[b].T).astype(BFNP)
        ctxT = context[b].T.astype(BFNP).astype(np.float32)
        # ctxpk[sup*128+p, k*512+c] = ctxT[k*128+p, sup*512+c]
        C4 = ctxT.reshape(KT, 128, N_SUP, CTX_SUP)
        ctxpk = np.ascontiguousarray(
            C4.transpose(2, 1, 0, 3).reshape(N_SUP * 128,
                                             KT * CTX_SUP)).astype(BFNP)
        m = dict(common)
        m.update({"ctxpk": ctxpk, "xoffpk": np.ascontiguousarray(xoffpk),
                  "xattpk": np.ascontiguousarray(xattpk)})
        in_maps.append(m)
    return in_maps


def kernel(**inputs):
    if "nc" not in _CACHE:
        _CACHE["nc"] = _build()
    nc = _CACHE["nc"]
    in_maps = _prep_inputs(inputs)
    res = run_bass_kernel_spmd(nc, in_maps, list(range(8)))
    _CACHE["last_results"] = res
    out = np.stack([res.results[i]["outT"].T for i in range(B)], axis=0)
    return np.ascontiguousarray(out.astype(np.float32))
